# revision 1
# baseline (speedup 1.0000x reference)
# Self-contained Trainium2 Bass kernel for the LN->QKV->sparse-rel-pos-attention->proj block.
#
# Reference computation (B=128, N=256, DIM=512, H=12, KD=32, D=128):
#   xn   = LayerNorm(x) * gamma + beta
#   qkv  = xn @ Wqkv + bqkv ; split q,k,v per head
#   attn = softmax(q k^T / sqrt(KD) + biases[:, bias_idxs])
#   out  = (attn @ v) @ Wproj + bproj
#
# Strategy: pure data-parallel over batch across 8 NeuronCores (16 elems/core).
# Host folds: gamma/beta into Wqkv, 1/sqrt(KD) into Wq, v-bias into bproj,
# and expands exp(biases[:, bias_idxs]) so softmax(S+B) = expS*expB row-normalized.
# Device layouts avoid all transposes except the initial z -> z^T:
#   qk^T [feat, tok] and v [tok, feat] both come from matmuls against z^T;
#   S^T = k q^T has tokens-m on partitions so exp/Z/AV consume it directly;
#   AV gives O^T [head-dim, tok] which is exactly proj's stationary layout.
# Softmax normalizer: Z^T = P^T-colsum via ones-matmul, reciprocal on DVE,
# PE-transpose to a row, DMA partition-broadcast, one fused DVE multiply.

import numpy as np

B, N, DIM = 128, 256, 512
H, KD = 12, 32
D = 128
DH = D * H
RES = 16
EPS = 1e-5
NCORES = 8
BPC = B // NCORES

_CACHE = {}

# heads processed in strip-pure pairs: strips (h % 3) equal within each pair
HEAD_ORDER = [0, 3, 6, 9, 1, 4, 7, 10, 2, 5, 8, 11]


def _build(bpc, use_bqk, use_bp, mmdt="float32r"):
    from contextlib import ExitStack

    import concourse.bacc as bacc
    import concourse.tile as tile
    from concourse import mybir
    from concourse.masks import make_identity

    f32 = mybir.dt.float32
    f32r = getattr(mybir.dt, mmdt)
    Alu = mybir.AluOpType
    Act = mybir.ActivationFunctionType

    nc = bacc.Bacc("TRN2", target_bir_lowering=False, debug=False,
                   num_devices=NCORES)

    x_d = nc.dram_tensor("x", [bpc, N, DIM], f32, kind="ExternalInput").ap()
    wqk_d = nc.dram_tensor("wqk", [DIM, 8 * 128], f32r, kind="ExternalInput").ap()
    wv_d = nc.dram_tensor("wv", [DIM, DH], f32r, kind="ExternalInput").ap()
    wp_d = nc.dram_tensor("wp", [DH, DIM], f32r, kind="ExternalInput").ap()
    expb_d = nc.dram_tensor("expb", [128, 2, H, N], f32, kind="ExternalInput").ap()
    ones_d = nc.dram_tensor("ones", [128, 1], f32r, kind="ExternalInput").ap()
    if use_bqk:
        bqk_d = nc.dram_tensor("bqk", [128, 8], f32, kind="ExternalInput").ap()
    if use_bp:
        bp_d = nc.dram_tensor("bp", [DIM], f32, kind="ExternalInput").ap()
    y_d = nc.dram_tensor("y", [bpc, N, DIM], f32, kind="ExternalOutput").ap()

    with tile.TileContext(nc) as tc, ExitStack() as ctx:
        consts = ctx.enter_context(tc.tile_pool(name="consts", bufs=1))
        sb_x = ctx.enter_context(tc.tile_pool(name="sb_x", bufs=2))
        sb_zT = ctx.enter_context(tc.tile_pool(name="sb_zT", bufs=2))
        sb_qkT = ctx.enter_context(tc.tile_pool(name="sb_qkT", bufs=2))
        sb_v = ctx.enter_context(tc.tile_pool(name="sb_v", bufs=2))
        sb_pt = ctx.enter_context(tc.tile_pool(name="sb_pt", bufs=2))
        sb_zb = ctx.enter_context(tc.tile_pool(name="sb_zb", bufs=2))
        sb_ot = ctx.enter_context(tc.tile_pool(name="sb_ot", bufs=2))
        sb_small = ctx.enter_context(tc.tile_pool(name="sb_small", bufs=3))
        ps_work = ctx.enter_context(tc.tile_pool(name="ps_work", bufs=2, space="PSUM"))
        ps_s = ctx.enter_context(tc.tile_pool(name="ps_s", bufs=2, space="PSUM"))
        ps_ot = ctx.enter_context(tc.tile_pool(name="ps_ot", bufs=2, space="PSUM"))
        ps_z = ctx.enter_context(tc.tile_pool(name="ps_z", bufs=1, space="PSUM"))
        ps_y = ctx.enter_context(tc.tile_pool(name="ps_y", bufs=1, space="PSUM"))
        dram = ctx.enter_context(tc.tile_pool(name="dram", bufs=2, space="DRAM"))

        # ---- constants ----
        wqk_sb = consts.tile([128, 4, 8 * 128], f32r)
        nc.sync.dma_start(out=wqk_sb, in_=wqk_d.rearrange("(kc p) f -> p kc f", p=128))
        wv_sb = consts.tile([128, 4, DH], f32r)
        nc.sync.dma_start(out=wv_sb, in_=wv_d.rearrange("(kc p) f -> p kc f", p=128))
        wp_sb = consts.tile([128, H, DIM], f32r)
        nc.sync.dma_start(out=wp_sb, in_=wp_d.rearrange("(h p) f -> p h f", p=128))
        expb_sb = consts.tile([128, 2, H, N], f32)
        nc.sync.dma_start(out=expb_sb, in_=expb_d)
        ident = consts.tile([128, 128], f32)
        make_identity(nc, ident)
        ones_col = consts.tile([128, 1], f32r)
        nc.sync.dma_start(out=ones_col, in_=ones_d)
        eps_t = consts.tile([128, 1], f32)
        nc.vector.memset(eps_t, EPS)
        if use_bqk:
            bqk_sb = consts.tile([128, 8], f32)
            nc.sync.dma_start(out=bqk_sb, in_=bqk_d)
        if use_bp:
            bp_sb = consts.tile([128, 1, DIM], f32)
            nc.sync.dma_start(out=bp_sb, in_=bp_d.partition_broadcast(128))

        assert bpc % 2 == 0
        for ep in range(bpc // 2):
            # ---- LayerNorm (token-major) + PE transpose to z^T, elem pair --
            # all four bn-stats first so ACT runs a single Sqrt per pair
            # (table reloads between Sqrt and Exp are ~1.3us each)
            zT_sb = sb_zT.tile([128, 4, 2 * N], f32r, tag="zT")
            x_ts = []
            mv = sb_small.tile([128, 2, 2, 2], f32, tag="mv")
            for el in range(2):
                for tci in range(2):
                    x_t = sb_x.tile([128, DIM], f32, tag="x", bufs=5)
                    nc.sync.dma_start(
                        out=x_t,
                        in_=x_d[2 * ep + el, tci * 128:(tci + 1) * 128, :])
                    stats = sb_small.tile([128, 6], f32, tag="stats")
                    nc.vector.bn_stats(stats, x_t)
                    nc.vector.bn_aggr(mv[:, el, tci, :], stats)
                    x_ts.append(x_t)
            sig = sb_small.tile([128, 2, 2], f32, tag="sig")
            nc.scalar.activation(sig, mv[:, :, :, 1], Act.Sqrt, bias=eps_t,
                                 scale=1.0)
            rsig = sb_small.tile([128, 2, 2], f32, tag="rsig")
            nc.vector.reciprocal(rsig, sig)
            for el in range(2):
                for tci in range(2):
                    x_t = x_ts[2 * el + tci]
                    nc.vector.tensor_scalar(out=x_t, in0=x_t,
                                            scalar1=mv[:, el, tci, 0:1],
                                            scalar2=rsig[:, el, tci:tci + 1],
                                            op0=Alu.subtract, op1=Alu.mult)
                    zT_ps = ps_work.tile([128, 512], f32, tag="work")
                    for kc in range(4):
                        nc.tensor.transpose(zT_ps[:, kc * 128:(kc + 1) * 128],
                                            x_t[:, kc * 128:(kc + 1) * 128],
                                            ident)
                    off = el * N + tci * 128
                    nc.scalar.activation(zT_sb[:, :, off:off + 128],
                                         zT_ps.rearrange("p (kc t) -> p kc t",
                                                         kc=4),
                                         Act.Copy)

            # ---- qk^T = W'' ^T z^T   [feat, tok-pair].  Head h's q lives in
            # chunk h//3, its k in chunk 4 + h//3, both at 32-row strip h%3.
            # The S matmuls contract K=32 at partition base 32*(h%3); heads
            # are processed in strip-pure pairs because interleaving different
            # PE tile_positions on one PSUM bank hangs the device
            # (sem-separated bank reuse across strips is fine).
            qkT_sb = sb_qkT.tile([128, 8, 2 * N], f32r, tag="qkT", bufs=1)
            for fc in range(8):
                qk_ps = ps_work.tile([128, 512], f32, tag="work")
                for kc in range(4):
                    nc.tensor.matmul(qk_ps,
                                     lhsT=wqk_sb[:, kc, fc * 128:(fc + 1) * 128],
                                     rhs=zT_sb[:, kc, :],
                                     start=(kc == 0), stop=(kc == 3))
                nc.scalar.activation(qkT_sb[:, fc, :], qk_ps, Act.Copy)
                if use_bqk:
                    nc.vector.tensor_scalar_add(
                        out=qkT_sb[:, fc, :], in0=qkT_sb[:, fc, :],
                        scalar1=bqk_sb[:, fc:fc + 1])

            for el in range(2):
                e = 2 * ep + el
                etok = el * N
                # ---- v = z Wv   [tok 256, feat 1536] ----
                v_sb = sb_v.tile([128, 2, DH], f32r, tag="v")
                for mc in range(2):
                    for ns in range(3):
                        v_ps = ps_work.tile([128, 512], f32, tag="work")
                        for kc in range(4):
                            nc.tensor.matmul(
                                v_ps,
                                lhsT=zT_sb[:, kc,
                                           etok + mc * 128:etok + (mc + 1) * 128],
                                rhs=wv_sb[:, kc, ns * 512:(ns + 1) * 512],
                                start=(kc == 0), stop=(kc == 3))
                        nc.vector.tensor_copy(
                            out=v_sb[:, mc, ns * 512:(ns + 1) * 512], in_=v_ps)

                # ---- attention, strip-pure head pairs ----
                # slot 2g+hl in pt/ot/expb corresponds to HEAD_ORDER[2g+hl]
                ot_sb = sb_ot.tile([128, H, N], f32r, tag="ot")
                for g in range(6):
                    pt_sb = sb_pt.tile([128, 2, 2, N], f32r, tag="pt")
                    for mc in range(2):
                        s_ps = ps_s.tile([128, 512], f32, tag="s")
                        for hl in range(2):
                            h = HEAD_ORDER[2 * g + hl]
                            qc = h // 3
                            base = (h % 3) * KD
                            nc.tensor.matmul(
                                s_ps[:, hl * N:(hl + 1) * N],
                                lhsT=qkT_sb[base:base + KD, 4 + qc,
                                            etok + mc * 128:etok + (mc + 1) * 128],
                                rhs=qkT_sb[base:base + KD, qc, etok:etok + N],
                                start=True, stop=True)
                        nc.scalar.activation(pt_sb[:, mc],
                                             s_ps.rearrange("p (a n) -> p a n",
                                                            a=2),
                                             Act.Exp)
                        # alternate the expB multiply between GpSimd and DVE
                        eng = nc.gpsimd if (g + mc) % 2 == 0 else nc.vector
                        eng.tensor_tensor(out=pt_sb[:, mc], in0=pt_sb[:, mc],
                                          in1=expb_sb[:, mc, 2 * g:2 * g + 2, :],
                                          op=Alu.mult)
                    # Z[hl, n] = sum_m P^T[m, n]; one matmul per mc chunk
                    zrow_ps = ps_z.tile([1, 512], f32, tag="zrow")
                    for mc in range(2):
                        nc.tensor.matmul(zrow_ps,
                                         lhsT=ones_col,
                                         rhs=pt_sb[:, mc, :, :].rearrange(
                                             "p a n -> p (a n)"),
                                         start=(mc == 0), stop=(mc == 1))
                    zrecip_sb = sb_small.tile([1, 512], f32, tag="zrecip")
                    nc.vector.reciprocal_approx_fast(out=zrecip_sb, in_=zrow_ps)
                    # partition-broadcast via DRAM roundtrip (step-0 partition
                    # APs are only legal on DRAM sources)
                    zscr = dram.tile([1, 512], f32, tag="zscr")
                    nc.sync.dma_start(out=zscr, in_=zrecip_sb)
                    zb_sb = sb_zb.tile([128, 2, N], f32, tag="zb")
                    nc.sync.dma_start(out=zb_sb,
                                      in_=zscr[0, :].partition_broadcast(128))
                    # O^T = v^T P^T  (normalized by zb afterwards)
                    ot_ps = ps_ot.tile([128, 512], f32, tag="otp")
                    for hl in range(2):
                        h = HEAD_ORDER[2 * g + hl]
                        for mc in range(2):
                            nc.tensor.matmul(
                                ot_ps[:, hl * N:(hl + 1) * N],
                                lhsT=v_sb[:, mc, h * 128:(h + 1) * 128],
                                rhs=pt_sb[:, mc, hl, :],
                                start=(mc == 0), stop=(mc == 1))
                    nc.vector.tensor_tensor(
                        out=ot_sb[:, 2 * g:2 * g + 2, :],
                        in0=ot_ps.rearrange("p (a n) -> p a n", a=2),
                        in1=zb_sb, op=Alu.mult)

                # ---- proj: y = O Wp ----
                for nci in range(2):
                    y_ps = ps_y.tile([128, 512], f32, tag="y")
                    for slot in range(H):
                        nc.tensor.matmul(y_ps,
                                         lhsT=ot_sb[:, slot, nci * 128:(nci + 1) * 128],
                                         rhs=wp_sb[:, HEAD_ORDER[slot], :],
                                         start=(slot == 0), stop=(slot == H - 1))
                    yb_sb = sb_x.tile([128, DIM], f32, tag="yb")
                    if use_bp:
                        nc.vector.tensor_tensor(out=yb_sb, in0=y_ps, in1=bp_sb[:, 0, :],
                                                op=Alu.add)
                    else:
                        nc.scalar.activation(yb_sb, y_ps, Act.Copy)
                    nc.sync.dma_start(out=y_d[e, nci * 128:(nci + 1) * 128, :],
                                      in_=yb_sb)

    nc.compile()
    return nc


def _prepare(x, gamma, beta, Wqkv, bqkv, Wproj, bproj, biases, bias_idxs):
    x = np.ascontiguousarray(np.asarray(x, dtype=np.float32))
    gamma = np.asarray(gamma, dtype=np.float32)
    beta = np.asarray(beta, dtype=np.float32)
    Wqkv = np.asarray(Wqkv, dtype=np.float32)
    bqkv = np.asarray(bqkv, dtype=np.float32)
    Wproj = np.asarray(Wproj, dtype=np.float32)
    bproj = np.asarray(bproj, dtype=np.float32)
    biases = np.asarray(biases, dtype=np.float32)
    bias_idxs = np.asarray(bias_idxs)

    s = np.float32(KD ** -0.5)
    Wg = Wqkv * gamma[:, None]
    bfull = beta @ Wqkv + bqkv
    Wr = Wg.reshape(DIM, H, 64 + D)
    br = bfull.reshape(H, 64 + D)
    # feature layout (see kernel comment): head h -> strip h%3; q in chunk
    # h//3, k in chunk 4 + h//3.
    wqk = np.zeros((DIM, 8, 128), dtype=np.float32)
    bqk = np.zeros((8, 128), dtype=np.float32)
    for h in range(H):
        qc, base = h // 3, (h % 3) * KD
        wqk[:, qc, base:base + KD] = Wr[:, h, 0:KD] * s
        wqk[:, 4 + qc, base:base + KD] = Wr[:, h, KD:2 * KD]
        bqk[qc, base:base + KD] = br[h, 0:KD] * s
        bqk[4 + qc, base:base + KD] = br[h, KD:2 * KD]
    wqk = np.ascontiguousarray(wqk.reshape(DIM, 8 * 128))
    wv = np.ascontiguousarray(Wr[:, :, 2 * KD:].reshape(DIM, DH))
    bv = br[:, 2 * KD:].reshape(DH)
    bp = bproj + bv @ Wproj
    expb = np.exp(biases[:, bias_idxs])  # [H, N, N]
    # head dim reordered to the kernel's strip-pure processing order
    expb_t = np.ascontiguousarray(
        expb[HEAD_ORDER].reshape(H, 2, 128, N).transpose(2, 1, 0, 3))

    use_bqk = bool(np.abs(bqk).max() > 0)
    use_bp = bool(np.abs(bp).max() > 0)
    bqk_t = np.ascontiguousarray(bqk.T)  # [128, 8]

    common = {"wqk": wqk, "wv": wv, "wp": np.ascontiguousarray(Wproj),
              "expb": expb_t, "ones": np.ones((128, 1), dtype=np.float32)}
    if use_bqk:
        common["bqk"] = bqk_t
    if use_bp:
        common["bp"] = np.ascontiguousarray(bp)
    in_maps = []
    for c in range(NCORES):
        m = dict(common)
        m["x"] = np.ascontiguousarray(x[c * BPC:(c + 1) * BPC])
        in_maps.append(m)
    return in_maps, use_bqk, use_bp


def run(inputs, trace=False, mmdt="float32r", **run_kwargs):
    from concourse.bass_utils import run_bass_kernel_spmd

    in_maps, use_bqk, use_bp = _prepare(**inputs)
    if mmdt == "float16":
        for m in in_maps:
            for k in ("wqk", "wv", "wp", "ones"):
                m[k] = m[k].astype(np.float16)
    key = (BPC, use_bqk, use_bp, mmdt)
    if key not in _CACHE:
        _CACHE[key] = _build(*key)
    nc = _CACHE[key]
    res = run_bass_kernel_spmd(nc, in_maps, core_ids=list(range(NCORES)),
                               trace=trace, **run_kwargs)
    y = np.concatenate([res.results[c]["y"] for c in range(NCORES)], axis=0)
    return y, res


def kernel(**inputs):
    y, _ = run(inputs)
    return y

